# revision 3
# baseline (speedup 1.0000x reference)
"""Causal shaped attention kernel for Trainium2 (8 NeuronCores).

y = beta * softmax(causal(q k^T / 8)) @ v + alpha * Id @ v - gamma * MC @ v
  with q,k = x @ w_attn.T split, v = x, Id = softmax(eye(T)), MC = causal row-mean.

Sharding: (batch, head-group) across 8 cores: core c -> b = c//2, heads
h0 = (c%2)*8 .. h0+8.  Each core computes y[b, :, h0*64 : h0*64+512].

Id@v and MC@v have closed forms (no T x T materialization):
  Id@v[i] = ((e-1) v[i] + colsum(v)) / (e+T-1)
  MC@v[i] = cumsum(v)[i] / (i+1)

On-device layout (per core):
  xT   [128, 8, 2048]   x[b]^T by 128-wide c-chunks (PE-transposed)
  WTq  [128, 4, 8, 128] per head-pair p, c-chunk ci: [Wq_even^T | Wq_odd^T]
  WTk  same for k
  qkT  [128, 4, 2, 2048] pair p: partitions 0:64 even head, 64:128 odd head;
                         [.., 0, :] = q^T, [.., 1, :] = k^T
  vones [128, 8, 16, 65] per head hh, j-tile J: [v | 1]  (AV lhsT)
  static [128, 16, 512]  k1*v + k2*colsum - gamma*cumsum/(i+1), combine addend

Attention per (head, i-strip g of 512): S^T tiles [j=128, i<=512] via PE
(K=64, fp32r), exp on ACT (scale=1/8 folded in), causal diag masked by tril
multiply, AV matmul lhsT=[v|1] gives y^T and rowsum in one pass, PE transpose
back, normalize + add static, DMA out.
"""

import sys

if "/opt/trn_rl_repo" not in sys.path:
    sys.path.insert(0, "/opt/trn_rl_repo")

import math

import numpy as np

import concourse.bass as bass
import concourse.mybir as mybir
import concourse.tile as tile
from concourse import bacc
from concourse.bass_utils import run_bass_kernel_spmd

F32 = mybir.dt.float32
F32R = mybir.dt.float32r
AF = mybir.ActivationFunctionType
OP = mybir.AluOpType

N_CORES = 8
B, T, C = 4, 2048, 1024
H, HD = 16, 64
NHC = 8          # heads per core
NT = T // 128    # 16 j/i tiles
NS = 4           # i-strips of 512
CONSTS_W = 276   # 128 tril + 16 negipg + k1 + k2 + kb + pad + 128 ident

_NC_CACHE = {}


def r(ap):
    return ap.bitcast(F32R)


def emit(nc, tc, xb, wqk, consts, yout):
    ctx_pools = []

    def pool(name, **kw):
        p = tc.alloc_tile_pool(name=name, **kw)
        ctx_pools.append(p)
        return p

    cpool = pool("cpool", bufs=1)
    ps = pool("ps", bufs=2, space="PSUM")

    cons = cpool.tile([128, CONSTS_W], F32, name="cons")
    nc.sync.dma_start(out=cons[:], in_=consts[:])
    tril = cons[:, 0:128]
    ident = cons[:, 148:276]
    negipg = cons[:, 128:144]      # [128, 16] : -gamma/(i+1)
    k1c = cons[:, 144:145]
    k2c = cons[:, 145:146]
    kbc = cons[:, 146:147]
    trilr = cpool.tile([128, 128], F32R, name="trilr")
    nc.vector.tensor_copy(out=trilr[:], in_=tril)
    ones_row = trilr[0:1, 0:128]   # tril row 0 == all ones (K=1 lhsT)
    ones_col = trilr[:, 127:128]   # tril col 127 == all ones [128, 1]

    qkp = pool("qkp", bufs=1)
    qkT = qkp.tile([128, 4, 2, 2048], F32R, name="qkT")

    # ---------------- phase A: transposes of W and x ----------------
    wtp = pool("wtp", bufs=1)
    WTq = wtp.tile([128, 4, 8, 128], F32R, name="WTq")
    WTk = wtp.tile([128, 4, 8, 128], F32R, name="WTk")
    xT = wtp.tile([128, 8, 2048], F32R, name="xT")

    ldp = pool("ldp", bufs=2)
    for p in range(4):
        for qk, WT in ((0, WTq), (1, WTk)):
            tw = ldp.tile([128, 1024], F32, name="tw", tag="tw")
            nc.sync.dma_start(out=tw[:], in_=wqk[qk * 512 + p * 128: qk * 512 + (p + 1) * 128, :])
            for cg in range(2):  # groups of 4 c-chunks
                pst = ps.tile([128, 512], F32, name="pst", tag="ps")
                for k in range(4):
                    ci = cg * 4 + k
                    nc.tensor.transpose(pst[:, k * 128:(k + 1) * 128],
                                        tw[:, ci * 128:(ci + 1) * 128], ident)
                nc.scalar.copy(out=WT[:, p, cg * 4:(cg + 1) * 4, :], in_=pst[:])
    for tt in range(NT):
        tx = ldp.tile([128, 1024], F32, name="tx", tag="tx")
        nc.sync.dma_start(out=tx[:], in_=xb[tt * 128:(tt + 1) * 128, :])
        for cg in range(2):
            pst = ps.tile([128, 512], F32, name="pstx", tag="ps")
            for k in range(4):
                ci = cg * 4 + k
                nc.tensor.transpose(pst[:, k * 128:(k + 1) * 128],
                                    tx[:, ci * 128:(ci + 1) * 128], ident)
            nc.scalar.copy(out=xT[:, cg * 4:(cg + 1) * 4, tt * 128:(tt + 1) * 128],
                           in_=pst[:].rearrange("p (a b) -> p a b", a=4))

    # ---------------- phase B: projections -> qkT ----------------
    for p in range(4):
        for qk, WT in ((0, WTq), (1, WTk)):
            for s in range(NS):
                pj = ps.tile([128, 512], F32, name="pj", tag="ps")
                for ci in range(8):
                    nc.tensor.matmul(pj[:], r(WT[:, p, ci, :]),
                                     r(xT[:, ci, s * 512:(s + 1) * 512]),
                                     start=(ci == 0), stop=(ci == 7))
                nc.vector.tensor_copy(out=qkT[:, p, qk, s * 512:(s + 1) * 512], in_=pj[:])

    # ---------------- phase B2: vones, colsum/cumsum, static ----------------
    ldp.release()
    ctx_pools.remove(ldp)
    wtp.release()
    ctx_pools.remove(wtp)
    b2 = pool("b2", bufs=1)
    b2s = pool("b2s", bufs=1)
    vones = b2.tile([128, NHC, NT, 65], F32R, name="vones")
    # strided gather of v columns: vones[p, hh, J, d] = xb[J*128+p, hh*64+d]
    nc.vector.memset(vones[:].bitcast(F32), 1.0)
    for hh in range(NHC):
        xs_view = xb[:, hh * 64:(hh + 1) * 64].rearrange("(J p) d -> p J d", p=128)
        nc.sync.dma_start(out=vones[:, hh, :, 0:64], in_=xs_view.bitcast(F32R))

    colb = b2.tile([128, 512], F32, name="colb")
    run = b2.tile([1, 512], F32R, name="run")       # exclusive prefix of tile colsums
    runs = b2.tile([1, 512], F32, name="runs")      # k2-scaled total (staging)
    static = b2.tile([128, NT, 512], F32, name="static")

    # pass 1: total colsum -> colb
    nc.vector.memset(run[:].bitcast(F32), 0.0)
    for I in range(NT):
        cp = ps.tile([1, 512], F32, name="cp", tag="cs", bufs=1)
        for hh in range(NHC):
            nc.tensor.matmul(cp[0:1, hh * 64:(hh + 1) * 64], r(ones_col),
                             r(vones[:, hh, I, 0:64]), start=True, stop=True)
        nc.vector.tensor_add(run[0:1, :], run[0:1, :], cp[0:1, :])
    nc.vector.tensor_scalar(out=runs[:], in0=run[0:1, :].bitcast(F32),
                            scalar1=cons[0:1, 145:146], scalar2=None, op0=OP.mult)
    nc.gpsimd.partition_broadcast(colb[:], runs[0:1, :])

    # pass 2: running exclusive prefix + cumsum + static
    nc.vector.memset(run[:].bitcast(F32), 0.0)
    for I in range(NT):
        cu = ps.tile([128, 512], F32, name="cu", tag="ps")
        nc.tensor.matmul(cu[:], r(ones_row), r(run[0:1, :]), start=True, stop=False)
        for hh in range(NHC):
            nc.tensor.matmul(cu[:, hh * 64:(hh + 1) * 64], r(trilr[:]),
                             r(vones[:, hh, I, 0:64]), start=False,
                             stop=(hh == NHC - 1))
        cp = ps.tile([1, 512], F32, name="cp2", tag="cs", bufs=1)
        for hh in range(NHC):
            nc.tensor.matmul(cp[0:1, hh * 64:(hh + 1) * 64], r(ones_col),
                             r(vones[:, hh, I, 0:64]), start=True, stop=True)
        nc.vector.tensor_add(run[0:1, :], run[0:1, :], cp[0:1, :])
        nc.vector.scalar_tensor_tensor(
            out=static[:, I, :].rearrange("p (h d) -> p h d", h=NHC),
            in0=vones[:, :, I, 0:64],
            scalar=k1c, in1=colb[:].rearrange("p (h d) -> p h d", h=NHC),
            op0=OP.mult, op1=OP.add)
        nc.vector.scalar_tensor_tensor(
            out=static[:, I, :], in0=cu[:], scalar=negipg[:, I:I + 1],
            in1=static[:, I, :], op0=OP.mult, op1=OP.add)

    # ---------------- phase C: attention per (head, i-strip) ----------------
    cp3 = pool("cp3", bufs=1)
    ptA = cp3.tile([128, 8, 512], F32R, name="ptA")
    ptB = cp3.tile([128, 8, 512], F32R, name="ptB")
    ysp = pool("ysp", bufs=2)

    for p in range(4):
        for half in range(2):
            hh = 2 * p + half
            base = half * 64
            qT = qkT[base:base + 64, p, 0, :]
            kT = qkT[base:base + 64, p, 1, :]
            for g in range(NS):
                nj = 4 * g + 4
                yps = ps.tile([128, 512], F32, name="yps", tag="yps", bufs=2)
                pts = []
                sidx = hh * NS + g

                def ptof(J):
                    if nj <= 8:
                        return (ptA if sidx % 2 == 0 else ptB)[:, J, :]
                    return ptA[:, J, :] if J < 8 else ptB[:, J - 8, :]

                def pt2of(J):
                    if nj <= 8:
                        return (ptA if sidx % 2 == 0 else ptB)[:, J:J + 2, :]
                    return ptA[:, J:J + 2, :] if J < 8 else ptB[:, J - 8:J - 6, :]

                J = 0
                while J < nj:
                    if J + 1 <= 4 * g and J % 2 == 0:
                        # two full-width j-tiles: one 2-bank psum, one exp
                        st2 = ps.tile([128, 2, 512], F32, name="st2", tag="ps2", bufs=1)
                        for u in range(2):
                            nc.tensor.matmul(
                                st2[:, u, :], r(kT[:, (J + u) * 128:(J + u + 1) * 128]),
                                r(qT[:, g * 512:(g + 1) * 512]),
                                start=True, stop=True)
                        pt2 = pt2of(J)
                        nc.scalar.activation(out=pt2, in_=st2[:],
                                             func=AF.Exp, scale=0.125)
                        for u in range(2):
                            if J + u == 4 * g:
                                nc.gpsimd.tensor_mul(pt2[:, u, 0:128],
                                                     pt2[:, u, 0:128], tril)
                            pts.append((pt2[:, u, :], 0))
                        J += 2
                        continue
                    i_off = max(0, 128 * J - 512 * g)
                    st = ps.tile([128, 512], F32, name="st", tag="ps")
                    nc.tensor.matmul(
                        st[:, i_off:512], r(kT[:, J * 128:(J + 1) * 128]),
                        r(qT[:, g * 512 + i_off:(g + 1) * 512]),
                        start=True, stop=True)
                    pt = ptof(J)
                    nc.scalar.activation(out=pt[:, i_off:512], in_=st[:, i_off:512],
                                         func=AF.Exp, scale=0.125)
                    if i_off > 0 or J == 4 * g:
                        # diagonal tile: keep j <= i only
                        nc.gpsimd.tensor_mul(pt[:, i_off:i_off + 128],
                                             pt[:, i_off:i_off + 128], tril)
                    pts.append((pt, i_off))
                    J += 1
                for J in range(nj):
                    pt, i_off = pts[J]
                    nc.tensor.matmul(
                        yps[0:65, i_off:512], r(vones[:, hh, J, :]),
                        r(pt[:, i_off:512]),
                        start=(J == 0), stop=(J == nj - 1), skip_group_check=True)
                # evacuate y^T [65, 512], transpose back to [i, 65]
                ysb = ysp.tile([65, 512], F32, name="ysb", tag="ysb")
                nc.vector.tensor_copy(out=ysb[:], in_=yps[0:65, :])
                tp = ps.tile([128, 260], F32, name="tp", tag="tp", bufs=1)
                for k in range(4):
                    nc.tensor.transpose(tp[:, k * 65:(k + 1) * 65],
                                        ysb[:, k * 128:(k + 1) * 128], ident[0:65, 0:65])
                rc4 = ysp.tile([128, 4], F32, name="rc4", tag="rc4")
                nc.vector.reciprocal(out=rc4[:], in_=tp[:, 64:260:65])
                nc.vector.tensor_scalar(out=rc4[:], in0=rc4[:], scalar1=kbc,
                                        scalar2=None, op0=OP.mult)
                yo = ysp.tile([128, 4, 64], F32, name="yo", tag="yo")
                for k in range(4):
                    nc.vector.scalar_tensor_tensor(
                        out=yo[:, k, :], in0=tp[:, k * 65:k * 65 + 64],
                        scalar=rc4[:, k:k + 1],
                        in1=static[:, 4 * g + k, hh * 64:(hh + 1) * 64],
                        op0=OP.mult, op1=OP.add)
                nc.sync.dma_start(
                    out=yout[g * 512:(g + 1) * 512, hh * 64:(hh + 1) * 64]
                    .rearrange("(k p) d -> p k d", p=128),
                    in_=yo[:])

    for p in reversed(ctx_pools):
        p.release()


def build_nc():
    if "nc" in _NC_CACHE:
        return _NC_CACHE["nc"]
    nc = bacc.Bacc("TRN2", target_bir_lowering=False)
    xb = nc.declare_dram_parameter("xb", [T, C], F32, isOutput=False)
    wqk = nc.declare_dram_parameter("wqk", [C, C], F32, isOutput=False)
    consts = nc.declare_dram_parameter("consts", [128, CONSTS_W], F32, isOutput=False)
    yout = nc.declare_dram_parameter("yout", [T, 512], F32, isOutput=True)
    with tile.TileContext(nc) as tc:
        emit(nc, tc, xb, wqk, consts, yout)
    nc.compile()
    _NC_CACHE["nc"] = nc
    return nc


def make_consts(alpha, beta, gamma):
    D = math.e + T - 1
    k1 = alpha * (math.e - 1.0) / D
    k2 = alpha / D
    cons = np.zeros((128, CONSTS_W), dtype=np.float32)
    jj = np.arange(128)
    cons[:, 0:128] = (jj[:, None] <= jj[None, :]).astype(np.float32)  # tril mask
    for I in range(16):
        cons[:, 128 + I] = -gamma / (128.0 * I + jj + 1.0)
    cons[:, 144] = k1
    cons[:, 145] = k2
    cons[:, 146] = beta
    cons[:, 148:276] = np.eye(128, dtype=np.float32)
    return cons


def kernel(x, w_attn, alpha, beta, gamma, _trace=False, _tmpdir=None):
    x = np.asarray(x, dtype=np.float32)
    w_attn = np.asarray(w_attn, dtype=np.float32)
    alpha = float(np.asarray(alpha))
    beta = float(np.asarray(beta))
    gamma = float(np.asarray(gamma))

    nc = build_nc()
    cons = make_consts(alpha, beta, gamma)
    in_maps = []
    for c in range(N_CORES):
        b, h0 = c // 2, (c % 2) * 8
        wqk = np.concatenate(
            [w_attn[h0 * 64: h0 * 64 + 512], w_attn[C + h0 * 64: C + h0 * 64 + 512]], axis=0)
        # rotate columns of x and w so this core's v-block sits at columns 0:512
        # (the projection q,k = x @ w.T is invariant to a consistent column roll)
        c0 = h0 * 64
        xb_r = np.roll(x[b], -c0, axis=1)
        wqk_r = np.roll(wqk, -c0, axis=1)
        in_maps.append({"xb": np.ascontiguousarray(xb_r),
                        "wqk": np.ascontiguousarray(wqk_r), "consts": cons})
    res = run_bass_kernel_spmd(nc, in_maps, list(range(N_CORES)), trace=_trace,
                               tmpdir=_tmpdir)
    y = np.empty((B, T, C), dtype=np.float32)
    for c in range(N_CORES):
        b, h0 = c // 2, (c % 2) * 8
        y[b, :, h0 * 64: h0 * 64 + 512] = res.results[c]["yout"]
    if _trace:
        kernel.last_exec_time_ns = res.exec_time_ns
    return y



# revision 13
# speedup vs baseline: 1.4104x; 1.4104x over previous
"""Causal shaped attention kernel for Trainium2 (8 NeuronCores).

y = beta * softmax(causal(q k^T / 8)) @ v + alpha * Id @ v - gamma * MC @ v
  with q,k = x @ w_attn.T split, v = x, Id = softmax(eye(T)), MC = causal row-mean.

Sharding: (batch, head-group) across 8 cores: core c -> b = c//2, heads
h0 = (c%2)*8 .. h0+8.  Each core computes y[b, :, h0*64 : h0*64+512].

Id@v and MC@v have closed forms (no T x T materialization):
  Id@v[i] = ((e-1) v[i] + colsum(v)) / (e+T-1)
  MC@v[i] = cumsum(v)[i] / (i+1)

On-device layout (per core):
  xT    [128, 8, 2048] bf16  x[b]^T by 128-wide c-chunks (PE-transposed)
  WTq   [128, 4, 8, 128] bf16  per head-pair p, c-chunk ci: [Wq_even^T | Wq_odd^T]
  WTk   same for k
  qkT   [128, 4, 2, 2048] bf16  pair p: partitions 0:64 even head, 64:128 odd;
                          [.., 0, :] = q^T, [.., 1, :] = k^T
  vload [128, 16, 512] f32   v rows by 128-tile (B2 colsum/cumsum operand)
  vones [128, 8, 16, 65] bf16  per head hh, j-tile J: [v | 1]  (AV lhsT)
  static [128, 16, 512] f32  k1*v + k2*colsum - gamma*cumsum/(i+1) addend

Attention per (pair p, i-strip g of 512): even/odd heads' S^T j-tiles go to the
two banks of one PSUM tile via concurrent K=64 row-group matmuls
(tile_position (0,0)/(64,0)).  Causal masking is an additive -2400*triu
matmul accumulated into PSUM before the scores (exp scale 1/8 -> -300 -> 0).
One exp (ACT) per (J, both heads) -> bf16 pt.  AV per head: lhsT=[v|1] bf16
gives y^T and rowsum in one accumulation chain; PE transpose back (bf16),
normalize + add static, DMA out.
"""

import sys

if "/opt/trn_rl_repo" not in sys.path:
    sys.path.insert(0, "/opt/trn_rl_repo")

import math

import numpy as np

import concourse.bass as bass
import concourse.mybir as mybir
import concourse.tile as tile
from concourse import bacc
from concourse.bass_utils import run_bass_kernel_spmd

F32 = mybir.dt.float32
F32R = mybir.dt.float32r
BF16 = mybir.dt.bfloat16
AF = mybir.ActivationFunctionType
OP = mybir.AluOpType

N_CORES = 8
B, T, C = 4, 2048, 1024
H, HD = 16, 64
NHC = 8          # heads per core
NT = T // 128    # 16 j/i tiles
NS = 4           # i-strips of 512
# consts: 128 tril + 16 negipg + k1 + k2 + kb + pad + 128 ident + 128 triuneg
CONSTS_W = 404

_NC_CACHE = {}


def r(ap):
    return ap.bitcast(F32R)


def emit(nc, tc, xb, wqk, consts, yout):
    ctx_pools = []

    def pool(name, **kw):
        p = tc.alloc_tile_pool(name=name, **kw)
        ctx_pools.append(p)
        return p

    cpool = pool("cpool", bufs=1)
    ps = pool("ps", bufs=2, space="PSUM")

    cons = cpool.tile([128, CONSTS_W], F32, name="cons")
    nc.sync.dma_start(out=cons[:], in_=consts[:])
    tril = cons[:, 0:128]
    ident = cons[:, 148:276]
    triuneg = cons[:, 276:404]
    negipg = cons[:, 128:144]      # [128, 16] : -gamma/(i+1)
    k1c = cons[:, 144:145]
    k2c = cons[:, 145:146]
    kbc = cons[:, 146:147]
    trilb = cpool.tile([128, 128], BF16, name="trilb")
    nc.vector.tensor_copy(out=trilb[:], in_=tril)
    ones_row = trilb[0:1, 0:128]   # tril row 0 == all ones (K=1 lhsT)
    ones_col = trilb[:, 127:128]   # tril col 127 == all ones [128, 1]
    identb = cpool.tile([128, 128], BF16, name="identb")
    nc.vector.tensor_copy(out=identb[:], in_=ident)
    triunegb = cpool.tile([128, 128], BF16, name="triunegb")
    nc.vector.tensor_copy(out=triunegb[:], in_=triuneg)

    qkp = pool("qkp", bufs=1)
    qkT = qkp.tile([128, 4, 2, 2048], BF16, name="qkT")

    # ---------------- phase A: transposes of W and x (fp32r, evac-cast bf16) --
    wtp = pool("wtp", bufs=1)
    WTq = wtp.tile([128, 4, 8, 128], BF16, name="WTq")
    WTk = wtp.tile([128, 4, 8, 128], BF16, name="WTk")
    xT = wtp.tile([128, 8, 2048], BF16, name="xT")

    ldp = pool("ldp", bufs=2)
    for p in range(4):
        for qk, WT in ((0, WTq), (1, WTk)):
            tw = ldp.tile([128, 1024], F32, name="tw", tag="tw")
            nc.sync.dma_start(out=tw[:], in_=wqk[qk * 512 + p * 128: qk * 512 + (p + 1) * 128, :])
            for cg in range(2):  # groups of 4 c-chunks
                pst = ps.tile([128, 512], F32, name="pst", tag="ps")
                for k in range(4):
                    ci = cg * 4 + k
                    nc.tensor.transpose(pst[:, k * 128:(k + 1) * 128],
                                        tw[:, ci * 128:(ci + 1) * 128], ident)
                nc.scalar.copy(out=WT[:, p, cg * 4:(cg + 1) * 4, :], in_=pst[:])
    for tt in range(NT):
        tx = ldp.tile([128, 1024], F32, name="tx", tag="tx")
        nc.sync.dma_start(out=tx[:], in_=xb[tt * 128:(tt + 1) * 128, :])
        for cg in range(2):
            pst = ps.tile([128, 512], F32, name="pstx", tag="ps")
            for k in range(4):
                ci = cg * 4 + k
                nc.tensor.transpose(pst[:, k * 128:(k + 1) * 128],
                                    tx[:, ci * 128:(ci + 1) * 128], ident)
            nc.vector.tensor_copy(out=xT[:, cg * 4:(cg + 1) * 4, tt * 128:(tt + 1) * 128],
                                  in_=pst[:].rearrange("p (a b) -> p a b", a=4))

    # ---------------- phase B: projections -> qkT (bf16 matmuls) ----------------
    for p in range(4):
        for qk, WT in ((0, WTq), (1, WTk)):
            for s in range(NS):
                pj = ps.tile([128, 512], F32, name="pj", tag="ps")
                for ci in range(8):
                    nc.tensor.matmul(pj[:], WT[:, p, ci, :],
                                     xT[:, ci, s * 512:(s + 1) * 512],
                                     start=(ci == 0), stop=(ci == 7))
                nc.scalar.copy(out=qkT[:, p, qk, s * 512:(s + 1) * 512], in_=pj[:])

    # ---------------- phase B2: vones, colsum/cumsum, static ----------------
    ldp.release()
    ctx_pools.remove(ldp)
    wtp.release()
    ctx_pools.remove(wtp)
    b2 = pool("b2", bufs=1)
    vload = b2.tile([128, NT, 512], F32, name="vload")
    nc.sync.dma_start(out=vload[:],
                      in_=xb[:, 0:512].rearrange("(J p) d -> p J d", p=128))
    vb16 = b2.tile([128, NT, 512], BF16, name="vb16")
    nc.vector.tensor_copy(out=vb16[:], in_=vload[:])
    vones = b2.tile([128, NHC, NT, 65], BF16, name="vones")
    nc.vector.memset(vones[:], 1.0)
    for hh in range(NHC):
        nc.vector.tensor_copy(out=vones[:, hh, :, 0:64],
                              in_=vb16[:, :, hh * 64:(hh + 1) * 64])

    colb = b2.tile([128, 512], F32, name="colb")
    run = b2.tile([1, 512], BF16, name="run")       # exclusive prefix of tile colsums
    runs = b2.tile([1, 512], F32, name="runs")      # k2-scaled total (staging)
    static = b2.tile([128, NT, 512], F32, name="static")

    # pass 1: total colsum -> colb
    cp1 = ps.tile([1, 512], F32, name="cp1", tag="cs", bufs=1)
    for I in range(NT):
        nc.tensor.matmul(cp1[0:1, :], ones_col, vb16[:, I, :],
                         start=(I == 0), stop=(I == NT - 1))
    nc.vector.tensor_scalar(out=runs[:], in0=cp1[0:1, :],
                            scalar1=cons[0:1, 145:146], scalar2=None, op0=OP.mult)
    nc.gpsimd.partition_broadcast(colb[:], runs[0:1, :])

    # pass 2: running exclusive prefix + cumsum + static
    nc.vector.memset(run[:], 0.0)
    for I in range(NT):
        cu = ps.tile([128, 512], F32, name="cu", tag="ps")
        nc.tensor.matmul(cu[:], ones_row, run[0:1, :], start=True, stop=False)
        nc.tensor.matmul(cu[:], trilb[:], vb16[:, I, :], start=False, stop=True)
        cp = ps.tile([1, 512], F32, name="cp2", tag="cs", bufs=1)
        nc.tensor.matmul(cp[0:1, :], ones_col, vb16[:, I, :],
                         start=True, stop=True)
        nc.vector.tensor_add(run[0:1, :], run[0:1, :], cp[0:1, :])
        nc.vector.scalar_tensor_tensor(
            out=static[:, I, :].rearrange("p (h d) -> p h d", h=NHC),
            in0=vload[:, I, :].rearrange("p (h d) -> p h d", h=NHC),
            scalar=k1c, in1=colb[:].rearrange("p (h d) -> p h d", h=NHC),
            op0=OP.mult, op1=OP.add)
        nc.vector.scalar_tensor_tensor(
            out=static[:, I, :], in0=cu[:], scalar=negipg[:, I:I + 1],
            in1=static[:, I, :], op0=OP.mult, op1=OP.add)

    # ---------------- phase C: attention per (pair, i-strip) ----------------
    ps.release()
    ctx_pools.remove(ps)
    cp3 = pool("cp3", bufs=1)
    # pt[buf][:, h, J, :] : exp(S^T) for head h (0=even,1=odd), j-tile J
    ptbuf = [cp3.tile([128, 2, NT, 512], BF16, name=f"pt{i}") for i in range(2)]
    ysp = pool("ysp", bufs=2)
    psC = pool("psC", bufs=2, space="PSUM")
    psY = pool("psY", bufs=1, space="PSUM")

    for p in range(4):
        qe = qkT[0:64, p, 0, :]
        qo = qkT[64:128, p, 0, :]
        ke = qkT[0:64, p, 1, :]
        ko = qkT[64:128, p, 1, :]
        for g in range(NS):
            nj = 4 * g + 4
            sidx = p * NS + g
            pt = ptbuf[sidx % 2]
            gs = slice(g * 512, (g + 1) * 512)

            # ---- scores + exp per j-tile ----
            for J in range(nj):
                st2 = psC.tile([128, 2, 512], F32, name="st2", tag="st2")
                js = slice(J * 128, (J + 1) * 128)
                if J < 4 * g:
                    # full tile: packed concurrent K=64 row-group matmuls
                    nc.tensor.matmul(st2[:, 0, :], ke[:, js], qe[:, gs],
                                     start=True, stop=True, tile_position=(0, 0))
                    nc.tensor.matmul(st2[:, 1, :], ko[:, js], qo[:, gs],
                                     start=True, stop=True, tile_position=(64, 0))
                    nc.scalar.activation(out=pt[:, :, J, :], in_=st2[:],
                                         func=AF.Exp, scale=0.125)
                else:
                    # diagonal-region tile: additive -2400*triu mask in PSUM,
                    # then scores on top; region below i_off never read.
                    i_off = 128 * J - 512 * g
                    blk = slice(i_off, i_off + 128)
                    gb = slice(g * 512 + i_off, g * 512 + i_off + 128)
                    nc.tensor.matmul(st2[:, 0, blk], identb[:], triunegb[:],
                                     start=True, stop=False, skip_group_check=True)
                    nc.tensor.matmul(st2[:, 0, blk], ke[:, js], qe[:, gb],
                                     start=False, stop=True, tile_position=(0, 0),
                                     skip_group_check=True)
                    nc.tensor.matmul(st2[:, 1, blk], identb[:], triunegb[:],
                                     start=True, stop=False, skip_group_check=True)
                    nc.tensor.matmul(st2[:, 1, blk], ko[:, js], qo[:, gb],
                                     start=False, stop=True, tile_position=(64, 0),
                                     skip_group_check=True)
                    if i_off + 128 < 512:
                        rest = slice(i_off + 128, 512)
                        gr = slice(g * 512 + i_off + 128, (g + 1) * 512)
                        nc.tensor.matmul(st2[:, 0, rest], ke[:, js], qe[:, gr],
                                         start=True, stop=True, tile_position=(0, 0),
                                         skip_group_check=True)
                        nc.tensor.matmul(st2[:, 1, rest], ko[:, js], qo[:, gr],
                                         start=True, stop=True, tile_position=(64, 0),
                                         skip_group_check=True)
                    nc.scalar.activation(out=pt[:, :, J, i_off:512],
                                         in_=st2[:, :, i_off:512],
                                         func=AF.Exp, scale=0.125)

            # ---- AV + y-post per head ----
            for half in range(2):
                hh = 2 * p + half
                yps = psY.tile([128, 512], F32, name="yps", tag=f"yps{half}",
                               bufs=1)
                for J in range(nj):
                    i_off = max(0, 128 * J - 512 * g)
                    nc.tensor.matmul(
                        yps[0:65, i_off:512], vones[:, hh, J, :],
                        pt[:, half, J, i_off:512],
                        start=(J == 0), stop=(J == nj - 1), skip_group_check=True)
                # evacuate y^T [65, 512] (bf16), transpose back to [i, 65]
                ysb = ysp.tile([65, 512], BF16, name="ysb", tag="ysb")
                nc.vector.tensor_copy(out=ysb[:], in_=yps[0:65, :])
                tp = psY.tile([128, 4, 66], BF16, name="tp", tag="tp", bufs=1)
                for k in range(4):
                    nc.tensor.transpose(tp[:, k, 0:65],
                                        ysb[:, k * 128:(k + 1) * 128],
                                        identb[0:65, 0:65])
                rc4 = ysp.tile([128, 4], F32, name="rc4", tag="rc4")
                nc.vector.reciprocal(out=rc4[:], in_=tp[:, :, 64])
                nc.vector.tensor_scalar(out=rc4[:], in0=rc4[:], scalar1=kbc,
                                        scalar2=None, op0=OP.mult)
                yo = ysp.tile([128, 4, 64], F32, name="yo", tag="yo")
                for k in range(4):
                    nc.vector.scalar_tensor_tensor(
                        out=yo[:, k, :], in0=tp[:, k, 0:64],
                        scalar=rc4[:, k:k + 1],
                        in1=static[:, 4 * g + k, hh * 64:(hh + 1) * 64],
                        op0=OP.mult, op1=OP.add)
                nc.sync.dma_start(
                    out=yout[g * 512:(g + 1) * 512, hh * 64:(hh + 1) * 64]
                    .rearrange("(k p) d -> p k d", p=128),
                    in_=yo[:])

    for p in reversed(ctx_pools):
        p.release()


def build_nc():
    if "nc" in _NC_CACHE:
        return _NC_CACHE["nc"]
    nc = bacc.Bacc("TRN2", target_bir_lowering=False)
    xb = nc.declare_dram_parameter("xb", [T, C], F32, isOutput=False)
    wqk = nc.declare_dram_parameter("wqk", [C, C], F32, isOutput=False)
    consts = nc.declare_dram_parameter("consts", [128, CONSTS_W], F32, isOutput=False)
    yout = nc.declare_dram_parameter("yout", [T, 512], F32, isOutput=True)
    with tile.TileContext(nc) as tc:
        emit(nc, tc, xb, wqk, consts, yout)
    nc.compile()
    _NC_CACHE["nc"] = nc
    return nc


def make_consts(alpha, beta, gamma):
    D = math.e + T - 1
    k1 = alpha * (math.e - 1.0) / D
    k2 = alpha / D
    cons = np.zeros((128, CONSTS_W), dtype=np.float32)
    jj = np.arange(128)
    cons[:, 0:128] = (jj[:, None] <= jj[None, :]).astype(np.float32)  # tril mask
    for I in range(16):
        cons[:, 128 + I] = -gamma / (128.0 * I + jj + 1.0)
    cons[:, 144] = k1
    cons[:, 145] = k2
    cons[:, 146] = beta
    cons[:, 148:276] = np.eye(128, dtype=np.float32)
    # strict upper triangle (j > i): -2400 (exp scale 1/8 -> -300 -> exp = 0)
    cons[:, 276:404] = np.where(jj[:, None] > jj[None, :], -2400.0, 0.0)
    return cons


def kernel(x, w_attn, alpha, beta, gamma, _trace=False, _tmpdir=None):
    x = np.asarray(x, dtype=np.float32)
    w_attn = np.asarray(w_attn, dtype=np.float32)
    alpha = float(np.asarray(alpha))
    beta = float(np.asarray(beta))
    gamma = float(np.asarray(gamma))

    nc = build_nc()
    cons = make_consts(alpha, beta, gamma)
    in_maps = []
    for c in range(N_CORES):
        b, h0 = c // 2, (c % 2) * 8
        wqk = np.concatenate(
            [w_attn[h0 * 64: h0 * 64 + 512], w_attn[C + h0 * 64: C + h0 * 64 + 512]], axis=0)
        # rotate columns of x and w so this core's v-block sits at columns 0:512
        # (the projection q,k = x @ w.T is invariant to a consistent column roll)
        c0 = h0 * 64
        xb_r = np.roll(x[b], -c0, axis=1)
        wqk_r = np.roll(wqk, -c0, axis=1)
        in_maps.append({"xb": np.ascontiguousarray(xb_r),
                        "wqk": np.ascontiguousarray(wqk_r), "consts": cons})
    res = run_bass_kernel_spmd(nc, in_maps, list(range(N_CORES)), trace=_trace,
                               tmpdir=_tmpdir)
    y = np.empty((B, T, C), dtype=np.float32)
    for c in range(N_CORES):
        b, h0 = c // 2, (c % 2) * 8
        y[b, :, h0 * 64: h0 * 64 + 512] = res.results[c]["yout"]
    if _trace:
        kernel.last_exec_time_ns = res.exec_time_ns
    return y
